# revision 1
# baseline (speedup 1.0000x reference)
"""Differential attention kernel for Trainium2 (8 NeuronCores).

Sharding: 2 batches x 4 V-dim shards (8192 -> 2048 per core). Each core
computes its batch's full attention maps (cheap) and its 2048-wide slice of
V / out_proj; host sums the 4 partial out-projections per batch.

All matmuls run as float32r (fp32 storage, full-rate PE mode). Softmax is
computed without max-subtraction (scores are bounded: |s*scale| < ~15, safe
in fp32). The 1/sum(e1) normalizer is folded into the final out-proj PSUM
eviction (everything after the differential combine is linear, and it is a
per-q diagonal), so the combine is just p = e1 - e2 * (lam*s1/s2).
bqkv is applied on-device via the ScalarE bias port; bv/bo are folded in
exactly on the host using sum_k(diff_attn[q,:]) == 1 - lambda.
"""

import math

import numpy as np

import concourse.bass as bass
from concourse import bacc
import concourse.mybir as mybir
import concourse.tile as tile
from concourse import bass_utils
from concourse.bass import ts, ds
from concourse.masks import make_identity

# Problem shapes (hardcoded per harness contract).
B = 2
S = 2048
D = 512
DQK = 256            # width of each of Q1/Q2/K1/K2
VDIM = 8192
DM = 512             # output dim
NV = 4               # v-shards
VS = VDIM // NV      # 2048 per core
P = 128
QC = 512             # q-chunk
SCALE = 1.0 / math.sqrt(64.0)
LAMBDA_INIT = 0.8
LAYER_INDEX = 0

F32 = mybir.dt.float32
F32R = mybir.dt.float32r
EXP = mybir.ActivationFunctionType.Exp
IDENT = mybir.ActivationFunctionType.Identity
AXX = mybir.AxisListType.X

KD = D // P          # 4 contraction chunks of the input dim
MQ = (2 * D) // P    # 8 m-chunks of qkv output dim
SN = S // 512        # 4 free chunks of S
NKC = S // P         # 16 k-chunks of 128
NVC = VS // P        # 16 v-chunks of 128
NQC = S // QC        # 4 q-chunks
QB = QC // P         # 4 q-blocks per chunk


def kernel_body(tc, xT, wqkv, wv, wo, lam, bq, out, phases="full"):
    nc = tc.nc
    # tolerate f32-typed dram tensors (e.g. run_kernel's sim harness)
    if xT.dtype != F32R:
        xT = xT.bitcast(F32R)
    if wqkv.dtype != F32R:
        wqkv = wqkv.bitcast(F32R)
    if wv.dtype != F32R:
        wv = wv.bitcast(F32R)
    if wo.dtype != F32R:
        wo = wo.bitcast(F32R)
    with (
        tc.tile_pool(name="persist", bufs=1) as persist,
        tc.tile_pool(name="dram", bufs=1, space="DRAM") as dram,
    ):
        _kernel_inner(tc, nc, persist, dram, xT, wqkv, wv, wo, lam, bq, out, phases)


def _kernel_inner(tc, nc, persist, dram, xT, wqkv, wv, wo, lam, bq, out, phases="full"):
    qkvT = persist.tile([P, MQ, S], F32R)     # [d|m-chunks, s]; m: Q1,Q1,Q2,Q2,K1,K1,K2,K2
    lam_sb = persist.tile([P, 1], F32)
    bq_sb = persist.tile([P, MQ], F32)
    ident_f32 = persist.tile([P, P], F32)
    ident = persist.tile([P, P], F32R)

    nc.sync.dma_start(lam_sb, lam)
    nc.sync.dma_start(bq_sb, bq)
    make_identity(nc, ident_f32)
    nc.vector.tensor_copy(ident, ident_f32)

    vd = dram.tile([NVC, P, NKC, P], F32R)  # V panels in [vc][k_in, k_out, v] layout

    # ---------------- setup: qkvT and V ----------------
    with (
        tc.tile_pool(name="setup", bufs=1) as setup,
        tc.tile_pool(name="sbounce", bufs=1) as sbounce,
        tc.tile_pool(name="spsum", bufs=6, space="PSUM") as spsum,
    ):
        xTs = setup.tile([P, KD, S], F32R)
        wq_sb = setup.tile([P, KD, 2 * D], F32R)
        wv_sb = setup.tile([P, KD, VS], F32R)
        # split input loads finely, in first-consumption order: the first
        # psum group (sn=0) needs wq m-half 0 of every dc and xTs[:, dc, sn0]
        for dc in range(KD):
            nc.sync.dma_start(wq_sb[:, dc, :D], wqkv[ds(dc * P, P), :D])
            nc.sync.dma_start(xTs[:, dc, ts(0, 512)], xT[ds(dc * P, P), ts(0, 512)])
        for dc in range(KD):
            nc.sync.dma_start(wq_sb[:, dc, D:], wqkv[ds(dc * P, P), D:])
        for sn in range(1, SN):
            for dc in range(KD):
                nc.sync.dma_start(xTs[:, dc, ts(sn, 512)],
                                  xT[ds(dc * P, P), ts(sn, 512)])
        for dc in range(KD):
            nc.sync.dma_start(wv_sb[:, dc], wv[ds(dc * P, P), :])

        # qkvT[m*128+p, s] = sum_d Wqkv[d, m*128+p] * xT[d, s] + bqkv
        for sn in range(SN):
            for m in range(MQ):
                pt = spsum.tile([P, 512], F32, tag="ps")
                for dc in range(KD):
                    nc.tensor.matmul(
                        pt, wq_sb[:, dc, ts(m, P)], xTs[:, dc, ts(sn, 512)],
                        start=(dc == 0), stop=(dc == KD - 1))
                nc.scalar.activation(qkvT[:, m, ts(sn, 512)], pt, IDENT,
                                     bias=bq_sb[:, m : m + 1])
        # V[s, v] = sum_d x[s, d] Wv[d, v]; staged 4 k-blocks at a time so
        # the panel-layout DRAM writes use 2KB per-partition lines
        NVN = VS // 512
        for g in range(S // P // 4):
            stages = [sbounce.tile([P, 4, 512], F32R, tag=f"st{vn}",
                                   name=f"st_{g}_{vn}") for vn in range(NVN)]
            for smj in range(4):
                sm = g * 4 + smj
                for vn in range(NVN):
                    pt = spsum.tile([P, 512], F32, tag="ps")
                    for dc in range(KD):
                        nc.tensor.matmul(
                            pt, xTs[:, dc, ts(sm, P)], wv_sb[:, dc, ts(vn, 512)],
                            start=(dc == 0), stop=(dc == KD - 1))
                    nc.vector.tensor_copy(stages[vn][:, smj], pt)
            for vn in range(NVN):
                for j in range(4):
                    nc.sync.dma_start(
                        vd[vn * 4 + j, :, ds(g * 4, 4), :],
                        stages[vn][:, :, ts(j, P)])

    if phases == "setup":
        # debug consumer to defeat DCE
        dbg = persist.tile([P, 512], F32, name="dbg")
        nc.vector.tensor_copy(dbg, qkvT[:, 0, :512])
        nc.sync.dma_start(out[ds(0, P), :], dbg)
        nc.sync.dma_start(out[ds(P, P), :].bitcast(F32R), vd[0, :, 0, :].broadcast_to((P, 512)) if False else vd[0].rearrange("p kc v -> p (kc v)")[:, :512])
        return

    # ---------------- main: attention per q-chunk ----------------
    with (
        tc.tile_pool(name="e1p", bufs=2) as e1p,
        tc.tile_pool(name="e2p", bufs=2) as e2p,
        tc.tile_pool(name="tmpp", bufs=4) as tmpp,
        tc.tile_pool(name="smallp", bufs=3) as smallp,
        tc.tile_pool(name="r1p", bufs=2 * QB) as r1p,
        tc.tile_pool(name="ptp", bufs=1) as ptp,
        tc.tile_pool(name="vpp", bufs=3) as vpp,
        tc.tile_pool(name="otp", bufs=3) as otp,
        tc.tile_pool(name="ofp", bufs=2) as ofp,
        tc.tile_pool(name="wop", bufs=1) as wop,
        tc.tile_pool(name="wps", bufs=4, space="PSUM") as wps,
        tc.tile_pool(name="fps", bufs=4, space="PSUM") as fps,
    ):
        woT = wop.tile([P, NVC, DM], F32R)
        nc.sync.dma_start(woT, wo.rearrange("(vc p) m -> p vc m", p=P))
        for qi in range(NQC):
            ptile = ptp.tile([P, NKC, QC], F32R, tag="pt")
            r1s = []
            pend = []   # deferred combine+transpose work, one entry per qb

            def emit_scores(qb):
                qg = qi * QB + qb
                ets = []
                sums = []
                for mi in range(2):
                    qoff, koff = 2 * mi, 4 + 2 * mi
                    pool = e1p if mi == 0 else e2p
                    et = pool.tile([P, S], F32R, tag=f"e{mi}", name=f"e{mi}_{qi}_{qb}")
                    st = smallp.tile([P, SN], F32, tag=f"sum{mi}",
                                     name=f"sum{mi}_{qi}_{qb}")
                    for kn in range(SN):
                        pt = wps.tile([P, 512], F32, tag="ps", name=f"ps_{qi}_{qb}_{mi}_{kn}")
                        for dc in range(2):
                            nc.tensor.matmul(
                                pt,
                                qkvT[:, qoff + dc, ts(qg, P)],
                                qkvT[:, koff + dc, ts(kn, 512)],
                                start=(dc == 0), stop=(dc == 1))
                        nc.scalar.activation(
                            et[:, ts(kn, 512)], pt, EXP, scale=SCALE,
                            accum_out=st[:, kn : kn + 1])
                    ets.append(et)
                    sums.append(st)
                # normalizers: r1 = 1/s1 folded into final out-proj evict;
                # combine uses r2q = lam * s1 / s2.
                s1 = smallp.tile([P, 1], F32, tag="s1", name=f"s1_{qi}_{qb}")
                nc.vector.reduce_sum(s1, sums[0], axis=AXX)
                r1 = r1p.tile([P, 1], F32, tag="r1", name=f"r1_{qi}_{qb}")
                nc.vector.reciprocal(r1, s1)
                r1s.append(r1)
                s2 = smallp.tile([P, 1], F32, tag="s2", name=f"s2_{qi}_{qb}")
                nc.vector.reduce_sum(s2, sums[1], axis=AXX)
                r2 = smallp.tile([P, 1], F32, tag="r2", name=f"r2_{qi}_{qb}")
                nc.vector.reciprocal(r2, s2)
                u = smallp.tile([P, 1], F32, tag="u", name=f"u_{qi}_{qb}")
                nc.vector.tensor_mul(u, s1, lam_sb)
                r2q = smallp.tile([P, 1], F32, tag="r2q", name=f"r2q_{qi}_{qb}")
                nc.vector.tensor_mul(r2q, u, r2)
                pend.append((qb, ets, r2q))

            def emit_combine():
                qb, ets, r2q = pend.pop(0)
                # p = e1 - e2 * r2q   (into ets[0]); t on ACT, sub on DVE
                for kn in range(SN):
                    ks = ts(kn, 512)
                    t2 = tmpp.tile([P, 512], F32, tag="t2", name=f"t2_{qi}_{qb}_{kn}")
                    nc.vector.tensor_scalar_mul(t2, ets[1][:, ks], r2q)
                    nc.vector.tensor_sub(ets[0][:, ks], ets[0][:, ks], t2)
                # transpose p into ptile[:, :, qb-block]; batch 4 transposes
                # into one psum bank, evict with one strided copy
                for kc4 in range(NKC // 4):
                    tp = wps.tile([P, 4, P], F32R, tag="ps", name=f"tp_{qi}_{qb}_{kc4}")
                    for j in range(4):
                        kc = kc4 * 4 + j
                        nc.tensor.matmul(tp[:, j], ets[0][:, ts(kc, P)], ident,
                                         is_transpose=True)
                    nc.vector.tensor_copy(ptile[:, ts(kc4, 4), ts(qb, P)], tp)

            # software pipeline: scores(qb+1) sits ahead of combine(qb) in the
            # PE queue so the PE never stalls on the ACT/DVE combine tail
            for qb in range(QB):
                emit_scores(qb)
                if qb > 0:
                    emit_combine()
            emit_combine()

            if phases == "scores":
                dbg2 = ofp.tile([P, DM], F32, tag="of", name=f"dbg2_{qi}")
                nc.vector.tensor_copy(dbg2, ptile[:, 0, :DM].bitcast(F32))
                nc.sync.dma_start(out[ds(qi * QC, P), :], dbg2)
                continue

            # attn @ V (transposed out) and out-proj, accumulated over v-chunks;
            # same trick: oT(vc+1) accumulation is queued before outF(vc)
            fts = [fps.tile([P, DM], F32, tag="f", name=f"f_{qi}_{q}") for q in range(QB)]
            ot_pend = []

            def emit_ot(vc):
                vp = vpp.tile([P, NKC, P], F32R, tag="vp", name=f"vp_{qi}_{vc}")
                nc.sync.dma_start(vp, vd[vc])
                ot_ps = wps.tile([P, 512], F32, tag="ps", name=f"otps_{qi}_{vc}")
                for kc in range(NKC):
                    nc.tensor.matmul(
                        ot_ps, vp[:, kc, :], ptile[:, kc, :],
                        start=(kc == 0), stop=(kc == NKC - 1))
                ot_sb = otp.tile([P, 512], F32R, tag="ot", name=f"ot_{qi}_{vc}")
                nc.vector.tensor_copy(ot_sb, ot_ps)
                ot_pend.append((vc, ot_sb))

            def emit_outf():
                vc, ot_sb = ot_pend.pop(0)
                for qs in range(QB):
                    nc.tensor.matmul(
                        fts[qs], ot_sb[:, ts(qs, P)], woT[:, vc, :],
                        start=(vc == 0), stop=(vc == NVC - 1))

            for vc in range(NVC):
                emit_ot(vc)
                if vc > 0:
                    emit_outf()
            emit_outf()
            for qs in range(QB):
                ofsb = ofp.tile([P, DM], F32, tag="of", name=f"of_{qi}_{qs}")
                nc.scalar.activation(ofsb, fts[qs], IDENT, scale=r1s[qs])
                nc.sync.dma_start(out[ds(qi * QC + qs * P, P), :], ofsb)


def build_module(n_iters=1, phases="full"):
    nc = bacc.Bacc("TRN2", target_bir_lowering=False, debug=False)
    xT = nc.dram_tensor("xT", (D, S), F32R, kind="ExternalInput").ap()
    wqkv = nc.dram_tensor("wqkv", (D, 2 * D), F32R, kind="ExternalInput").ap()
    wv = nc.dram_tensor("wv", (D, VS), F32R, kind="ExternalInput").ap()
    wo = nc.dram_tensor("wo", (VS, DM), F32R, kind="ExternalInput").ap()
    lam = nc.dram_tensor("lam", (P, 1), F32, kind="ExternalInput").ap()
    bq = nc.dram_tensor("bq", (P, MQ), F32, kind="ExternalInput").ap()
    out = nc.dram_tensor("out", (S, DM), F32, kind="ExternalOutput").ap()
    with tile.TileContext(nc) as tc:
        for _ in range(n_iters):
            kernel_body(tc, xT, wqkv, wv, wo, lam, bq, out, phases)
    nc.compile()
    return nc


_NC = None


def _get_module():
    global _NC
    if _NC is None:
        _NC = build_module()
    return _NC


def host_prep(**inputs):
    """Host-side input prep: returns (in_maps, lam, host_bias)."""
    x = np.asarray(inputs["x"], np.float32)
    Wqkv = np.asarray(inputs["Wqkv"], np.float32)
    bqkv = np.asarray(inputs["bqkv"], np.float32)
    Wv = np.asarray(inputs["Wv"], np.float32)
    bv = np.asarray(inputs["bv"], np.float32)
    Wo = np.asarray(inputs["Wo"], np.float32)
    bo = np.asarray(inputs["bo"], np.float32)
    lq1 = np.asarray(inputs["lq1"], np.float32)
    lk1 = np.asarray(inputs["lk1"], np.float32)
    lq2 = np.asarray(inputs["lq2"], np.float32)
    lk2 = np.asarray(inputs["lk2"], np.float32)

    lam = float(
        np.exp(np.sum(lq1 * lk1, dtype=np.float32))
        - np.exp(np.sum(lq2 * lk2, dtype=np.float32))
        + (LAMBDA_INIT - 0.6 * math.exp(-0.3 * LAYER_INDEX))
    )
    bq_host = np.ascontiguousarray(bqkv.reshape(MQ, P).T)
    lam_host = np.full((P, 1), lam, np.float32)

    in_maps = []
    for c in range(8):
        b, v = divmod(c, NV)
        in_maps.append({
            "xT": np.ascontiguousarray(x[b].T),
            "wqkv": np.ascontiguousarray(Wqkv),
            "wv": np.ascontiguousarray(Wv[:, v * VS : (v + 1) * VS]),
            "wo": np.ascontiguousarray(Wo[v * VS : (v + 1) * VS, :]),
            "lam": lam_host,
            "bq": bq_host,
        })
    # sum_k diff_attn[q, :] == 1 - lam exactly, so bv and bo fold into a
    # constant per-output-column correction.
    host_bias = ((1.0 - lam) * bv) @ Wo + bo
    return in_maps, lam, host_bias.astype(np.float32)


def kernel(**inputs):
    in_maps, _lam, host_bias = host_prep(**inputs)
    nc = _get_module()
    res = bass_utils.run_bass_kernel_spmd(nc, in_maps, core_ids=list(range(8)))
    out = np.zeros((B, S, DM), np.float32)
    for c in range(8):
        b, _v = divmod(c, NV)
        out[b] += res.results[c]["out"]
    out += host_bias
    return out



# revision 33
# speedup vs baseline: 2.9247x; 2.9247x over previous
"""Differential attention kernel for Trainium2 (8 NeuronCores).

Key restructuring vs the reference: since V = x @ Wv and everything after the
differential combine is linear,

    out = diff_attn @ V @ Wo = diff_attn @ x @ (Wv @ Wo)

so the 8192-wide V projection and out-projection collapse into a single
512x512 fused weight Wf = Wv @ Wo (host GEMM), cutting device FLOPs ~6x.

Sharding: 2 batches x 4 query-shards (512 queries per core). Each core
computes full K1/K2 for its batch (duplicated x4), its own Q slice, both
attention maps for its queries, T_i = e_i @ x (bf16), the differential
combine folded into per-row ACT scales, and F = U @ Wf. No cross-core
reduction: each core owns its output rows exactly.

Softmax is unnormalized (scores bounded, safe in fp32); 1/s1 and lam/s2 fold
into the PSUM evictions of T1/T2. bv/bo fold into a host-side constant via
sum_k diff_attn[q,:] == 1 - lambda.
"""

import math

import numpy as np
import ml_dtypes

import concourse.bass as bass
from concourse import bacc
import concourse.mybir as mybir
import concourse.tile as tile
from concourse import bass_utils
from concourse.bass import ts, ds
from concourse.masks import make_identity

# Problem shapes (hardcoded per harness contract).
B = 2
S = 2048
D = 512
NQSH = 4             # query shards per batch
QS = S // NQSH       # 512 queries per core
P = 128
SCALE = 1.0 / math.sqrt(64.0)
LAMBDA_INIT = 0.8
LAYER_INDEX = 0

F32 = mybir.dt.float32
F32R = mybir.dt.float32r
BF16 = mybir.dt.bfloat16
EXP = mybir.ActivationFunctionType.Exp
IDENT = mybir.ActivationFunctionType.Identity
AXX = mybir.AxisListType.X

KD = D // P          # 4 contraction chunks of the model dim
SN = S // 512        # 4 key chunks of 512
NKC = S // P         # 16 key chunks of 128
QB = QS // P         # 4 query blocks of 128 per core


def setup_weights(tc, persist, wqkv, wf, lam, bq):
    """One-time (weight-resident) setup: weight DMAs + identity tiles."""
    nc = tc.nc
    if wqkv.dtype != F32R:
        wqkv = wqkv.bitcast(F32R)
    if wf.dtype != F32R:
        wf = wf.bitcast(F32R)
    lam_sb = persist.tile([P, 1], F32, tag="lam")
    bq_sb = persist.tile([P, 8], F32, tag="bq")
    ident_f32 = persist.tile([P, P], F32, tag="idf")
    ident = persist.tile([P, P], F32R, tag="idr")
    ident_bf = persist.tile([P, P], BF16, tag="idb")
    wq_sb = persist.tile([P, KD, 2 * D], F32R, tag="wq")
    wf_sb = persist.tile([P, KD, D], F32R, tag="wf")

    nc.sync.dma_start(lam_sb, lam)
    nc.sync.dma_start(bq_sb, bq)
    make_identity(nc, ident_f32)
    nc.vector.tensor_copy(ident, ident_f32)
    nc.vector.tensor_copy(ident_bf, ident_f32)
    for dc in range(KD):
        nc.sync.dma_start(wq_sb[:, dc, :D], wqkv[ds(dc * P, P), :D])
    for dc in range(KD):
        nc.sync.dma_start(wq_sb[:, dc, D:], wqkv[ds(dc * P, P), D:])
    for dc in range(KD):
        nc.sync.dma_start(wf_sb[:, dc], wf[ds(dc * P, P), :])
    return dict(lam_sb=lam_sb, bq_sb=bq_sb, ident=ident, ident_bf=ident_bf,
                wq_sb=wq_sb, wf_sb=wf_sb)


def kernel_body(tc, pools, wts, it, xT, xTq, xb, out, prev_tail=None):
    nc = tc.nc
    if xT.dtype != F32R:
        xT = xT.bitcast(F32R)
    if xTq.dtype != F32R:
        xTq = xTq.bitcast(F32R)
    return _kernel_inner(tc, nc, pools, wts, it, xT, xTq, xb, out, prev_tail)


def _kernel_inner(tc, nc, pools, wts, it, xT, xTq, xb, out, prev_tail=None):
    persist, etp, vup, smallp, ofp, wps, tps, fps = pools
    lam_sb, bq_sb, ident, ident_bf = (
        wts["lam_sb"], wts["bq_sb"], wts["ident"], wts["ident_bf"])
    wq_sb, wf_sb = wts["wq_sb"], wts["wf_sb"]

    xTq_sb = persist.tile([P, KD, QS], F32R, tag="xTq")
    xTs = persist.tile([P, KD, S], F32R, tag="xTs")
    xbs = persist.tile([P, NKC, D], BF16, tag="xbs")

    qkvT_Q = persist.tile([P, KD, QS], F32R, tag="qQ")  # [d', mq, q]
    # per-kn K tiles and per-qb e/sum tiles: finer dep granularity lets the
    # next iteration's evictions start as soon as the matching readers of
    # THIS iteration are done (tile deps are whole-tile).
    qK = [persist.tile([P, KD, 512], F32R, tag=f"qK{kn}", name=f"qK{kn}")
          for kn in range(SN)]
    # two half-tiles per qb (kn 0-1 | kn 2-3) so the combine and the
    # transposes pipeline at half granularity (tile deps are whole-tile)
    e_sb = [[persist.tile([P, 2, S // 2], BF16, tag=f"e{qb}h{h}",
                          name=f"e{qb}h{h}") for h in range(2)]
            for qb in range(QB)]
    st_sb = [persist.tile([P, 2, SN], F32, tag=f"st{qb}", name=f"st{qb}")
             for qb in range(QB)]

    # input DMAs in first-consumption order
    for dc in range(KD):
        nc.sync.dma_start(xTq_sb[:, dc], xTq[ds(dc * P, P), :])
    for sn in range(SN):
        for dc in range(KD):
            nc.sync.dma_start(xTs[:, dc, ts(sn, 512)], xT[ds(dc * P, P), ts(sn, 512)])
    for kc in range(NKC):
        nc.sync.dma_start(xbs[:, kc], xb[ds(kc * P, P), :])

    # ---------------- projections ----------------
    # PSUM evictions of the projections run on DVE (tensor_scalar_add with
    # per-partition bias) so ACT stays a pure Exp stream during scores.
    # Q-proj: dc-outer so the first matmul only needs the first DMA chunks.
    qpts = [wps.tile([P, QS], F32, tag="ps", name=f"qp_{mq}") for mq in range(KD)]
    for dc in range(KD):
        for mq in range(KD):
            nc.tensor.matmul(qpts[mq], wq_sb[:, dc, ts(mq, P)], xTq_sb[:, dc],
                             start=(dc == 0), stop=(dc == KD - 1))
    for mq in range(KD):
        nc.vector.tensor_scalar_add(qkvT_Q[:, mq], qpts[mq], bq_sb[:, mq : mq + 1])

    def emit_kproj(kn, mk):
        pt = wps.tile([P, 512], F32, tag="ps", name=f"kp_{kn}_{mk}")
        for dc in range(KD):
            nc.tensor.matmul(pt, wq_sb[:, dc, ds(D + mk * P, P)],
                             xTs[:, dc, ts(kn, 512)],
                             start=(dc == 0), stop=(dc == KD - 1))
        nc.vector.tensor_scalar_add(qK[kn][:, mk, :], pt,
                                    bq_sb[:, 4 + mk : 4 + mk + 1])

    def emit_score(kn, qb, mi):
        pt = wps.tile([P, 512], F32, tag="ps", name=f"s_{kn}_{qb}_{mi}")
        for dc in range(2):
            nc.tensor.matmul(
                pt,
                qkvT_Q[:, 2 * mi + dc, ts(qb, P)],
                qK[kn][:, 2 * mi + dc, :],
                start=(dc == 0), stop=(dc == 1))
        nc.scalar.activation(
            e_sb[qb][kn // 2][:, mi, ts(kn % 2, 512)], pt, EXP, scale=SCALE,
            accum_out=st_sb[qb][:, mi, kn : kn + 1])

    # K-proj for chunk kn+1 spliced between score groups of chunk kn: the
    # proj matmuls give ACT slack to drain the exp stream (ACT is slower
    # than the 2 matmuls feeding each score psum).
    prev_units = list(prev_tail) if prev_tail else []
    for mk in range(KD):
        emit_kproj(0, mk)
    if prev_units:
        prev_units.pop(0)()
    for kn in range(SN - 1):
        for qb in range(QB):
            emit_kproj(kn + 1, qb)
            emit_score(kn, qb, 0)
            emit_score(kn, qb, 1)
            if kn == 0 and prev_units:
                prev_units.pop(0)()
    assert not prev_units

    # ---------------- per-query-block tail ----------------
    # Last score chunk folds into the qb pipeline: stage1(qb) starts right
    # after qb's own final exp, while later qbs' scores still stream.
    pend = []

    r1s = {}

    def emit_stage1a(qb):
        # DVE-only: normalizers + the differential combine at the e-level:
        # p = e1 - c*e2 with c = lam*s1/s2; 1/s1 folds into the F evict.
        s1 = smallp.tile([P, 1], F32, tag="s1", name=f"s1_{qb}")
        nc.vector.reduce_sum(s1, st_sb[qb][:, 0], axis=AXX)
        r1 = smallp.tile([P, 1], F32, tag="r1", name=f"r1_{qb}")
        nc.vector.reciprocal(r1, s1)
        r1s[qb] = r1
        s2 = smallp.tile([P, 1], F32, tag="s2", name=f"s2_{qb}")
        nc.vector.reduce_sum(s2, st_sb[qb][:, 1], axis=AXX)
        r2 = smallp.tile([P, 1], F32, tag="r2", name=f"r2_{qb}")
        nc.vector.reciprocal(r2, s2)
        u1 = smallp.tile([P, 1], F32, tag="u1", name=f"u1_{qb}")
        nc.vector.tensor_mul(u1, s1, r2)
        cc = smallp.tile([P, 1], F32, tag="cc", name=f"cc_{qb}")
        nc.vector.tensor_mul(cc, u1, lam_sb)
        for h in range(2):
            t2 = vup.tile([P, 2, 512], BF16, tag="t2", name=f"t2_{qb}_{h}")
            nc.vector.tensor_scalar_mul(
                t2, e_sb[qb][h][:, 1].rearrange("p (a b) -> p a b", b=512), cc)
            nc.vector.tensor_sub(
                e_sb[qb][h][:, 0].rearrange("p (a b) -> p a b", b=512),
                e_sb[qb][h][:, 0].rearrange("p (a b) -> p a b", b=512), t2)

    def emit_stage1b(qb):
        # transpose p into [k, q] (bf16), 8 tiles per psum bank, then
        # T^T[d, q] = sum_k x[k, d] p^T[k, q] directly (lhsT = x chunks,
        # free dim = 128 q — bf16 stays full rate below 256). Skips the
        # T-transpose + one psum eviction entirely.
        pT = etp.tile([P, NKC, P], BF16, tag="pT", name=f"pT_{qb}")
        for h in range(2):
            tp = wps.tile([P, 8, P], BF16, tag="ps", name=f"tp_{qb}_{h}")
            for j in range(8):
                nc.tensor.transpose(
                    tp[:, j], e_sb[qb][h][:, 0, ts(j, P)], ident_bf)
            nc.vector.tensor_copy(pT[:, ts(h, 8), :], tp)
        tt = tps.tile([P, KD, P], F32, tag="t", name=f"t_{qb}")
        for dc in range(KD):
            for kc in range(NKC):
                nc.tensor.matmul(tt[:, dc, :], xbs[:, kc, ts(dc, P)],
                                 pT[:, kc, :],
                                 start=(kc == 0), stop=(kc == NKC - 1))
        pend.append((qb, tt, r1s[qb]))

    def emit_stage2():
        qb, tt, r1 = pend.pop(0)
        # F = T @ Wf (lhsT = T^T chunks), scaled by 1/s1 on evict
        uT = vup.tile([P, KD, P], F32R, tag="uT", name=f"uT_{qb}")
        nc.vector.tensor_copy(uT, tt)
        ft = fps.tile([P, D], F32, tag="f", name=f"f_{qb}")
        for dc in range(KD):
            nc.tensor.matmul(ft, uT[:, dc, :], wf_sb[:, dc, :],
                             start=(dc == 0), stop=(dc == KD - 1))
        ofsb = ofp.tile([P, D], F32, tag="of", name=f"of_{qb}")
        nc.vector.tensor_scalar_mul(ofsb, ft, r1)
        nc.sync.dma_start(out[ds(qb * P, P), :], ofsb)

    # 2-group lead: stage1a (DVE combine) emits right after each qb's last
    # scores; the PE part (stage1b) trails by 2 groups so the combine's DVE
    # latency hides behind later qbs' score matmuls.
    for qb in range(QB):
        emit_score(SN - 1, qb, 0)
        emit_score(SN - 1, qb, 1)
        emit_stage1a(qb)
        if qb >= 2:
            emit_stage1b(qb - 2)
        if qb >= 3:
            emit_stage2()
    # the remaining tail units are deferred into the next iteration's
    # emission stream (spliced between its projection/score groups, where
    # their ACT/DVE dependencies are long satisfied)
    return [lambda: emit_stage1b(QB - 2), emit_stage2,
            lambda: emit_stage1b(QB - 1), emit_stage2, emit_stage2]


def build_module(n_iters=1):
    nc = bacc.Bacc("TRN2", target_bir_lowering=False, debug=False)
    xT = nc.dram_tensor("xT", (D, S), F32R, kind="ExternalInput").ap()
    xTq = nc.dram_tensor("xTq", (D, QS), F32R, kind="ExternalInput").ap()
    xb = nc.dram_tensor("xb", (S, D), BF16, kind="ExternalInput").ap()
    wqkv = nc.dram_tensor("wqkv", (D, 2 * D), F32R, kind="ExternalInput").ap()
    wf = nc.dram_tensor("wf", (D, D), F32R, kind="ExternalInput").ap()
    lam = nc.dram_tensor("lam", (P, 1), F32, kind="ExternalInput").ap()
    bq = nc.dram_tensor("bq", (P, 8), F32, kind="ExternalInput").ap()
    out = nc.dram_tensor("out", (QS, D), F32, kind="ExternalOutput").ap()
    with tile.TileContext(nc) as tc:
        with (
            tc.tile_pool(name="persist", bufs=1) as persist,
            tc.tile_pool(name="etp", bufs=2) as etp,
            tc.tile_pool(name="vup", bufs=2) as vup,
            tc.tile_pool(name="smallp", bufs=4) as smallp,
            tc.tile_pool(name="ofp", bufs=2) as ofp,
            tc.tile_pool(name="wps", bufs=4, space="PSUM") as wps,
            tc.tile_pool(name="tps", bufs=2, space="PSUM") as tps,
            tc.tile_pool(name="fps", bufs=2, space="PSUM") as fps,
        ):
            pools = (persist, etp, vup, smallp, ofp, wps, tps, fps)
            wts = setup_weights(tc, persist, wqkv, wf, lam, bq)
            tail = None
            for it in range(n_iters):
                tail = kernel_body(tc, pools, wts, it, xT, xTq, xb, out,
                                   prev_tail=tail)
            for u in tail:
                u()
    nc.compile()
    return nc


_NC = None


def _get_module():
    global _NC
    if _NC is None:
        _NC = build_module()
    return _NC


def host_prep(**inputs):
    """Host-side input prep: returns (in_maps, lam, host_bias)."""
    x = np.asarray(inputs["x"], np.float32)
    Wqkv = np.asarray(inputs["Wqkv"], np.float32)
    bqkv = np.asarray(inputs["bqkv"], np.float32)
    Wv = np.asarray(inputs["Wv"], np.float32)
    bv = np.asarray(inputs["bv"], np.float32)
    Wo = np.asarray(inputs["Wo"], np.float32)
    bo = np.asarray(inputs["bo"], np.float32)
    lq1 = np.asarray(inputs["lq1"], np.float32)
    lk1 = np.asarray(inputs["lk1"], np.float32)
    lq2 = np.asarray(inputs["lq2"], np.float32)
    lk2 = np.asarray(inputs["lk2"], np.float32)

    lam_v = float(
        np.exp(np.sum(lq1 * lk1, dtype=np.float32))
        - np.exp(np.sum(lq2 * lk2, dtype=np.float32))
        + (LAMBDA_INIT - 0.6 * math.exp(-0.3 * LAYER_INDEX))
    )
    bq_host = np.ascontiguousarray(bqkv.reshape(8, P).T)
    lam_host = np.full((P, 1), lam_v, np.float32)
    Wf = np.ascontiguousarray(Wv @ Wo)  # [512, 512] fused weight

    in_maps = []
    for c in range(8):
        b, qs = divmod(c, NQSH)
        xTb = np.ascontiguousarray(x[b].T)
        in_maps.append({
            "xT": xTb,
            "xTq": np.ascontiguousarray(xTb[:, qs * QS : (qs + 1) * QS]),
            "xb": np.ascontiguousarray(x[b]).astype(ml_dtypes.bfloat16),
            "wqkv": np.ascontiguousarray(Wqkv),
            "wf": Wf,
            "lam": lam_host,
            "bq": bq_host,
        })
    # sum_k diff_attn[q, :] == 1 - lam exactly, so bv and bo fold into a
    # constant per-output-column correction.
    host_bias = ((1.0 - lam_v) * bv) @ Wo + bo
    return in_maps, lam_v, host_bias.astype(np.float32)


def kernel(**inputs):
    in_maps, _lam, host_bias = host_prep(**inputs)
    nc = _get_module()
    res = bass_utils.run_bass_kernel_spmd(nc, in_maps, core_ids=list(range(8)))
    out = np.zeros((B, S, D), np.float32)
    for c in range(8):
        b, qs = divmod(c, NQSH)
        out[b, qs * QS : (qs + 1) * QS, :] = res.results[c]["out"]
    out += host_bias
    return out


# revision 36
# speedup vs baseline: 70.2151x; 24.0075x over previous
"""Differential attention kernel for Trainium2 (8 NeuronCores).

Key restructuring vs the reference: since V = x @ Wv and everything after the
differential combine is linear,

    out = diff_attn @ V @ Wo = diff_attn @ x @ (Wv @ Wo)

so the 8192-wide V projection and out-projection collapse into a single
512x512 fused weight Wf = Wv @ Wo (host GEMM), cutting device FLOPs ~6x.

Sharding: 2 batches x 4 query-shards (512 queries per core). Each core
computes full K1/K2 for its batch (duplicated x4), its own Q slice, both
attention maps for its queries, T_i = e_i @ x (bf16), the differential
combine folded into per-row ACT scales, and F = U @ Wf. No cross-core
reduction: each core owns its output rows exactly.

Softmax is unnormalized (scores bounded, safe in fp32); 1/s1 and lam/s2 fold
into the PSUM evictions of T1/T2. bv/bo fold into a host-side constant via
sum_k diff_attn[q,:] == 1 - lambda.
"""

import math

import numpy as np
import ml_dtypes

import concourse.bass as bass
from concourse import bacc
import concourse.mybir as mybir
import concourse.tile as tile
from concourse import bass_utils
from concourse.bass import ts, ds
from concourse.masks import make_identity

# Problem shapes (hardcoded per harness contract).
B = 2
S = 2048
D = 512
NQSH = 4             # query shards per batch
QS = S // NQSH       # 512 queries per core
P = 128
SCALE = 1.0 / math.sqrt(64.0)
LAMBDA_INIT = 0.8
LAYER_INDEX = 0

F32 = mybir.dt.float32
F32R = mybir.dt.float32r
BF16 = mybir.dt.bfloat16
EXP = mybir.ActivationFunctionType.Exp
IDENT = mybir.ActivationFunctionType.Identity
AXX = mybir.AxisListType.X

KD = D // P          # 4 contraction chunks of the model dim
SN = S // 512        # 4 key chunks of 512
NKC = S // P         # 16 key chunks of 128
QB = QS // P         # 4 query blocks of 128 per core


def setup_weights(tc, persist, wqkv, wf, lam, bq):
    """One-time (weight-resident) setup: weight DMAs + identity tiles."""
    nc = tc.nc
    if wqkv.dtype != F32R:
        wqkv = wqkv.bitcast(F32R)
    if wf.dtype != F32R:
        wf = wf.bitcast(F32R)
    lam_sb = persist.tile([P, 1], F32, tag="lam")
    bq_sb = persist.tile([P, 8], F32, tag="bq")
    ident_f32 = persist.tile([P, P], F32, tag="idf")
    ident = persist.tile([P, P], F32R, tag="idr")
    ident_bf = persist.tile([P, P], BF16, tag="idb")
    wq_sb = persist.tile([P, KD, 2 * D], F32R, tag="wq")
    wf_sb = persist.tile([P, KD, D], F32R, tag="wf")

    nc.sync.dma_start(lam_sb, lam)
    nc.sync.dma_start(bq_sb, bq)
    make_identity(nc, ident_f32)
    nc.vector.tensor_copy(ident, ident_f32)
    nc.vector.tensor_copy(ident_bf, ident_f32)
    for dc in range(KD):
        nc.sync.dma_start(wq_sb[:, dc, :D], wqkv[ds(dc * P, P), :D])
    for dc in range(KD):
        nc.sync.dma_start(wq_sb[:, dc, D:], wqkv[ds(dc * P, P), D:])
    for dc in range(KD):
        nc.sync.dma_start(wf_sb[:, dc], wf[ds(dc * P, P), :])
    return dict(lam_sb=lam_sb, bq_sb=bq_sb, ident=ident, ident_bf=ident_bf,
                wq_sb=wq_sb, wf_sb=wf_sb)


def kernel_body(tc, pools, wts, it, xT, xTq, xb, out, prev_tail=None):
    nc = tc.nc
    if xT.dtype != F32R:
        xT = xT.bitcast(F32R)
    if xTq.dtype != F32R:
        xTq = xTq.bitcast(F32R)
    return _kernel_inner(tc, nc, pools, wts, it, xT, xTq, xb, out, prev_tail)


def _kernel_inner(tc, nc, pools, wts, it, xT, xTq, xb, out, prev_tail=None):
    persist, etp, vup, smallp, ofp, wps, tps, fps = pools
    lam_sb, bq_sb, ident, ident_bf = (
        wts["lam_sb"], wts["bq_sb"], wts["ident"], wts["ident_bf"])
    wq_sb, wf_sb = wts["wq_sb"], wts["wf_sb"]

    xTq_sb = persist.tile([P, KD, QS], F32R, tag="xTq")
    xTs = persist.tile([P, KD, S], F32R, tag="xTs")
    xbs = persist.tile([P, NKC, D], BF16, tag="xbs")

    qkvT_Q = persist.tile([P, KD, QS], F32R, tag="qQ")  # [d', mq, q]
    # per-kn K tiles and per-qb e/sum tiles: finer dep granularity lets the
    # next iteration's evictions start as soon as the matching readers of
    # THIS iteration are done (tile deps are whole-tile).
    qK = [persist.tile([P, KD, 512], F32R, tag=f"qK{kn}", name=f"qK{kn}")
          for kn in range(SN)]
    # two half-tiles per qb (kn 0-1 | kn 2-3) so the combine and the
    # transposes pipeline at half granularity (tile deps are whole-tile)
    e_sb = [[persist.tile([P, 2, S // 2], BF16, tag=f"e{qb}h{h}",
                          name=f"e{qb}h{h}") for h in range(2)]
            for qb in range(QB)]
    st_sb = [persist.tile([P, 2, SN], F32, tag=f"st{qb}", name=f"st{qb}")
             for qb in range(QB)]

    # input DMAs in first-consumption order
    for dc in range(KD):
        nc.sync.dma_start(xTq_sb[:, dc], xTq[ds(dc * P, P), :])
    for sn in range(SN):
        for dc in range(KD):
            nc.sync.dma_start(xTs[:, dc, ts(sn, 512)], xT[ds(dc * P, P), ts(sn, 512)])
    for kc in range(NKC):
        nc.sync.dma_start(xbs[:, kc], xb[ds(kc * P, P), :])

    # ---------------- projections ----------------
    # PSUM evictions of the projections run on DVE (tensor_scalar_add with
    # per-partition bias) so ACT stays a pure Exp stream during scores.
    # Q-proj: dc-outer so the first matmul only needs the first DMA chunks.
    qpts = [wps.tile([P, QS], F32, tag="ps", name=f"qp_{mq}") for mq in range(KD)]
    for dc in range(KD):
        for mq in range(KD):
            nc.tensor.matmul(qpts[mq], wq_sb[:, dc, ts(mq, P)], xTq_sb[:, dc],
                             start=(dc == 0), stop=(dc == KD - 1))
    for mq in range(KD):
        nc.vector.tensor_scalar_add(qkvT_Q[:, mq], qpts[mq], bq_sb[:, mq : mq + 1])

    def emit_kproj(kn, mk):
        pt = wps.tile([P, 512], F32, tag="ps", name=f"kp_{kn}_{mk}")
        for dc in range(KD):
            nc.tensor.matmul(pt, wq_sb[:, dc, ds(D + mk * P, P)],
                             xTs[:, dc, ts(kn, 512)],
                             start=(dc == 0), stop=(dc == KD - 1))
        nc.vector.tensor_scalar_add(qK[kn][:, mk, :], pt,
                                    bq_sb[:, 4 + mk : 4 + mk + 1])

    def emit_score(kn, qb, mi):
        pt = wps.tile([P, 512], F32, tag="ps", name=f"s_{kn}_{qb}_{mi}")
        for dc in range(2):
            nc.tensor.matmul(
                pt,
                qkvT_Q[:, 2 * mi + dc, ts(qb, P)],
                qK[kn][:, 2 * mi + dc, :],
                start=(dc == 0), stop=(dc == 1))
        nc.scalar.activation(
            e_sb[qb][kn // 2][:, mi, ts(kn % 2, 512)], pt, EXP, scale=SCALE,
            accum_out=st_sb[qb][:, mi, kn : kn + 1])

    # K-proj for chunk kn+1 spliced between score groups of chunk kn: the
    # proj matmuls give ACT slack to drain the exp stream (ACT is slower
    # than the 2 matmuls feeding each score psum).
    psums = {}
    prev_units = list(prev_tail) if prev_tail else []
    for mk in range(KD):
        emit_kproj(0, mk)
    if prev_units:
        prev_units.pop(0)()
    for kn in range(SN - 1):
        for qb in range(QB):
            emit_kproj(kn + 1, qb)
            emit_score(kn, qb, 0)
            emit_score(kn, qb, 1)
            if kn == 0 and prev_units:
                prev_units.pop(0)()
            if kn == SN - 2:
                # partial sums over kn 0..2 (off the tail's critical path)
                for mi in range(2):
                    psu = smallp.tile([P, 1], F32, tag=f"psu{mi}",
                                      name=f"psu{mi}_{qb}")
                    nc.vector.reduce_sum(psu, st_sb[qb][:, mi, : SN - 1],
                                         axis=AXX)
                    psums[(qb, mi)] = psu
    assert not prev_units

    # ---------------- per-query-block tail ----------------
    # Last score chunk folds into the qb pipeline: stage1(qb) starts right
    # after qb's own final exp, while later qbs' scores still stream.
    pend = []

    r1s = {}

    def emit_stage1a(qb):
        # DVE-only: normalizers + the differential combine at the e-level:
        # p = e1 - c*e2 with c = lam*s1/s2; 1/s1 folds into the F evict.
        # Chain after the last exp: 2 adds + recip + fused cc, then combine.
        s1 = smallp.tile([P, 1], F32, tag="s1", name=f"s1_{qb}")
        nc.vector.tensor_add(s1, psums[(qb, 0)], st_sb[qb][:, 0, SN - 1 :])
        s2 = smallp.tile([P, 1], F32, tag="s2", name=f"s2_{qb}")
        nc.vector.tensor_add(s2, psums[(qb, 1)], st_sb[qb][:, 1, SN - 1 :])
        r2 = smallp.tile([P, 1], F32, tag="r2", name=f"r2_{qb}")
        nc.vector.reciprocal(r2, s2)
        u1 = smallp.tile([P, 1], F32, tag="u1", name=f"u1_{qb}")
        nc.vector.tensor_mul(u1, s1, r2)
        cc = smallp.tile([P, 1], F32, tag="cc", name=f"cc_{qb}")
        nc.vector.tensor_mul(cc, u1, lam_sb)
        for h in range(2):
            t2 = vup.tile([P, 2, 512], BF16, tag="t2", name=f"t2_{qb}_{h}")
            nc.vector.tensor_scalar_mul(
                t2, e_sb[qb][h][:, 1].rearrange("p (a b) -> p a b", b=512), cc)
            nc.vector.tensor_sub(
                e_sb[qb][h][:, 0].rearrange("p (a b) -> p a b", b=512),
                e_sb[qb][h][:, 0].rearrange("p (a b) -> p a b", b=512), t2)
        r1 = smallp.tile([P, 1], F32, tag="r1", name=f"r1_{qb}")
        nc.vector.reciprocal(r1, s1)
        r1s[qb] = r1

    def emit_stage1b(qb):
        # transpose p into [k, q] (bf16), 8 tiles per psum bank, then
        # T^T[d, q] = sum_k x[k, d] p^T[k, q] directly (lhsT = x chunks,
        # free dim = 128 q — bf16 stays full rate below 256). Skips the
        # T-transpose + one psum eviction entirely.
        pT = etp.tile([P, NKC, P], BF16, tag="pT", name=f"pT_{qb}")
        for h in range(2):
            tp = wps.tile([P, 8, P], BF16, tag="ps", name=f"tp_{qb}_{h}")
            for j in range(8):
                nc.tensor.transpose(
                    tp[:, j], e_sb[qb][h][:, 0, ts(j, P)], ident_bf)
            nc.vector.tensor_copy(pT[:, ts(h, 8), :], tp)
        tt = tps.tile([P, KD, P], F32, tag="t", name=f"t_{qb}")
        for dc in range(KD):
            for kc in range(NKC):
                nc.tensor.matmul(tt[:, dc, :], xbs[:, kc, ts(dc, P)],
                                 pT[:, kc, :],
                                 start=(kc == 0), stop=(kc == NKC - 1))
        pend.append((qb, tt, r1s[qb]))

    def emit_stage2():
        qb, tt, r1 = pend.pop(0)
        # F = T @ Wf (lhsT = T^T chunks), scaled by 1/s1 on evict
        uT = vup.tile([P, KD, P], F32R, tag="uT", name=f"uT_{qb}")
        nc.vector.tensor_copy(uT, tt)
        ft = fps.tile([P, D], F32, tag="f", name=f"f_{qb}")
        for dc in range(KD):
            nc.tensor.matmul(ft, uT[:, dc, :], wf_sb[:, dc, :],
                             start=(dc == 0), stop=(dc == KD - 1))
        ofsb = ofp.tile([P, D], F32, tag="of", name=f"of_{qb}")
        nc.vector.tensor_scalar_mul(ofsb, ft, r1)
        nc.sync.dma_start(out[ds(qb * P, P), :], ofsb)

    # 2-group lead: stage1a (DVE combine) emits right after each qb's last
    # scores; the PE part (stage1b) trails by 2 groups so the combine's DVE
    # latency hides behind later qbs' score matmuls.
    for qb in range(QB):
        emit_score(SN - 1, qb, 0)
        emit_score(SN - 1, qb, 1)
        emit_stage1a(qb)
        if qb >= 2:
            emit_stage1b(qb - 2)
        if qb >= 3:
            emit_stage2()
    # the remaining tail units are deferred into the next iteration's
    # emission stream (spliced between its projection/score groups, where
    # their ACT/DVE dependencies are long satisfied)
    return [lambda: emit_stage1b(QB - 2), emit_stage2,
            lambda: emit_stage1b(QB - 1), emit_stage2, emit_stage2]


def build_module(n_iters=1):
    nc = bacc.Bacc("TRN2", target_bir_lowering=False, debug=False)
    xT = nc.dram_tensor("xT", (D, S), F32R, kind="ExternalInput").ap()
    xTq = nc.dram_tensor("xTq", (D, QS), F32R, kind="ExternalInput").ap()
    xb = nc.dram_tensor("xb", (S, D), BF16, kind="ExternalInput").ap()
    wqkv = nc.dram_tensor("wqkv", (D, 2 * D), F32R, kind="ExternalInput").ap()
    wf = nc.dram_tensor("wf", (D, D), F32R, kind="ExternalInput").ap()
    lam = nc.dram_tensor("lam", (P, 1), F32, kind="ExternalInput").ap()
    bq = nc.dram_tensor("bq", (P, 8), F32, kind="ExternalInput").ap()
    out = nc.dram_tensor("out", (QS, D), F32, kind="ExternalOutput").ap()
    with tile.TileContext(nc) as tc:
        with (
            tc.tile_pool(name="persist", bufs=1) as persist,
            tc.tile_pool(name="etp", bufs=2) as etp,
            tc.tile_pool(name="vup", bufs=2) as vup,
            tc.tile_pool(name="smallp", bufs=4) as smallp,
            tc.tile_pool(name="ofp", bufs=2) as ofp,
            tc.tile_pool(name="wps", bufs=4, space="PSUM") as wps,
            tc.tile_pool(name="tps", bufs=2, space="PSUM") as tps,
            tc.tile_pool(name="fps", bufs=2, space="PSUM") as fps,
        ):
            pools = (persist, etp, vup, smallp, ofp, wps, tps, fps)
            wts = setup_weights(tc, persist, wqkv, wf, lam, bq)
            tail = None
            for it in range(n_iters):
                tail = kernel_body(tc, pools, wts, it, xT, xTq, xb, out,
                                   prev_tail=tail)
            for u in tail:
                u()
    nc.compile()
    return nc


_NC = None


def _get_module():
    global _NC
    if _NC is None:
        _NC = build_module()
    return _NC


def host_prep(**inputs):
    """Host-side input prep: returns (in_maps, lam, host_bias)."""
    x = np.asarray(inputs["x"], np.float32)
    Wqkv = np.asarray(inputs["Wqkv"], np.float32)
    bqkv = np.asarray(inputs["bqkv"], np.float32)
    Wv = np.asarray(inputs["Wv"], np.float32)
    bv = np.asarray(inputs["bv"], np.float32)
    Wo = np.asarray(inputs["Wo"], np.float32)
    bo = np.asarray(inputs["bo"], np.float32)
    lq1 = np.asarray(inputs["lq1"], np.float32)
    lk1 = np.asarray(inputs["lk1"], np.float32)
    lq2 = np.asarray(inputs["lq2"], np.float32)
    lk2 = np.asarray(inputs["lk2"], np.float32)

    lam_v = float(
        np.exp(np.sum(lq1 * lk1, dtype=np.float32))
        - np.exp(np.sum(lq2 * lk2, dtype=np.float32))
        + (LAMBDA_INIT - 0.6 * math.exp(-0.3 * LAYER_INDEX))
    )
    bq_host = np.ascontiguousarray(bqkv.reshape(8, P).T)
    lam_host = np.full((P, 1), lam_v, np.float32)
    Wf = np.ascontiguousarray(Wv @ Wo)  # [512, 512] fused weight

    in_maps = []
    for c in range(8):
        b, qs = divmod(c, NQSH)
        xTb = np.ascontiguousarray(x[b].T)
        in_maps.append({
            "xT": xTb,
            "xTq": np.ascontiguousarray(xTb[:, qs * QS : (qs + 1) * QS]),
            "xb": np.ascontiguousarray(x[b]).astype(ml_dtypes.bfloat16),
            "wqkv": np.ascontiguousarray(Wqkv),
            "wf": Wf,
            "lam": lam_host,
            "bq": bq_host,
        })
    # sum_k diff_attn[q, :] == 1 - lam exactly, so bv and bo fold into a
    # constant per-output-column correction.
    host_bias = ((1.0 - lam_v) * bv) @ Wo + bo
    return in_maps, lam_v, host_bias.astype(np.float32)


def kernel(**inputs):
    in_maps, _lam, host_bias = host_prep(**inputs)
    nc = _get_module()
    res = bass_utils.run_bass_kernel_spmd(nc, in_maps, core_ids=list(range(8)))
    out = np.zeros((B, S, D), np.float32)
    for c in range(8):
        b, qs = divmod(c, NQSH)
        out[b, qs * QS : (qs + 1) * QS, :] = res.results[c]["out"]
    out += host_bias
    return out
